# revision 28
# baseline (speedup 1.0000x reference)
"""Bucket-indexed spatially-varying (channel-shared) 5x5 convolution on 8 trn2 cores.

out[b,c,y,x] = sum_{i,j} pad(input)[b,c,y+i,x+j] * kernel_bank[buckets[b,y,x], i, j]

Strategy (data-parallel over batch, one image per core):
  * All-fp16 on-device operands (tolerance is 2e-2; this keeps ~2e-3).
  * Layout: partition dim = output row y (two 128-row tiles), free dims =
    (x-block, channel) with channels innermost (stride 1).
  * Per-pixel weight map built on HOST: wm2[y, t, 2x] = bank[buckets[y,x], t]
    duplicated in adjacent x-pairs. The weight operand AP
    [x (stride 2), c-pair (stride 0 broadcast), 2 (stride 1)] matches the
    x-patch operand elementwise while keeping a packed 16-bit innermost dim,
    which keeps the DVE in 2x perf mode.
  * Per tap: one big DVE tensor_tensor mult -> prod tile; the 25-tap
    accumulation runs on the PE as identity-weight matmuls accumulating in
    PSUM fp32 (start at t=0, stop at t=24). The scalar engine drains
    PSUM -> SBUF f16 and DMA writes the result out.
  * Tap multiplies are spread over three engines by real-silicon
    calibration (DVE 2389ns / Pool 8825ns per 4096-elem op; ACT 423ns per
    128-elem column op): 18 taps on DVE (~688us), dx==2 of each dy on Pool
    (5 taps, ~706us), and (dy=2, dx in {0,4}) on the scalar engine as
    per-column activation-scale ops (scale = per-partition fp32 weight,
    ~494us incl. PSUM drains). PE accumulation is ~700us; every engine sits
    within ~3% of the ceiling.
  * dy taps read row-shifted x tiles (partition shifts are impossible inside
    SBUF ops), loaded per-dy and double-buffered.
"""

import sys

sys.path.insert(0, "/opt/trn_rl_repo")

import numpy as np

B, C, H, W = 8, 128, 256, 256
K, NB = 5, 64
PAD = (K - 1) // 2  # 2
HP, WP = H + 2 * PAD, W + 2 * PAD  # 260, 260
N_CORES = 8
NT = K * K  # 25 taps

XW = 32  # x block width
NXB = W // XW  # 16
YT = H // 128  # 2
NSL = XW * C // 512  # 512-col PSUM slices per x-tile (8)
# taps (dy=2, dx=0) and (dy=2, dx=4) run on the scalar (ACT) engine as
# per-column scale ops (~423ns each measured on silicon); both sit in the
# same dy so they share one xs tile
T_ACT = (2 * K + 0, 2 * K + 4)

_CACHE = {}


def _build_nc():
    import concourse.bacc as bacc
    import concourse.mybir as mybir
    from concourse import tile

    f16 = mybir.dt.float16
    f32 = mybir.dt.float32
    Alu = mybir.AluOpType

    nc = bacc.Bacc(None)

    xp = nc.dram_tensor("xp", [HP, WP, C], f16, kind="ExternalInput")
    wm2 = nc.dram_tensor("wm2", [H, NT, 2 * W], f16, kind="ExternalInput")
    # fp32 weight planes for the ACT-engine taps (ACT scale APs must be fp32)
    wma = nc.dram_tensor("wma", [H, len(T_ACT), W], f32, kind="ExternalInput")
    ident = nc.dram_tensor("ident", [128, 128], f16, kind="ExternalInput")
    y_out = nc.dram_tensor("y", [H, W, C], f16, kind="ExternalOutput")

    with tile.TileContext(nc) as tc:
        with (
            tc.tile_pool(name="const", bufs=1) as cpool,
            tc.tile_pool(name="xs", bufs=3) as xpool,
            tc.tile_pool(name="kv", bufs=2) as kvpool,
            tc.tile_pool(name="prod", bufs=10) as ppool,
            tc.tile_pool(name="accs", bufs=2) as apool,
            tc.tile_pool(name="psum", bufs=1, space="PSUM") as pspool,
        ):
            id_sb = cpool.tile([128, 128], f16)
            nc.sync.dma_start(out=id_sb[:], in_=ident[:])

            for yt in range(YT):
                for xb in range(NXB):
                    x0 = xb * XW
                    kv2 = kvpool.tile([128, NT, 2 * XW], f16, tag="kv")
                    nc.sync.dma_start(
                        out=kv2[:],
                        in_=wm2[
                            yt * 128 : (yt + 1) * 128, :, 2 * x0 : 2 * (x0 + XW)
                        ],
                    )
                    wma_sb = kvpool.tile([128, len(T_ACT), XW], f32, tag="wma")
                    nc.sync.dma_start(
                        out=wma_sb[:],
                        in_=wma[yt * 128 : (yt + 1) * 128, :, x0 : x0 + XW],
                    )
                    # two 4-bank PSUM accumulators per x-tile -> half-tile
                    # granularity pipelining of PE accumulate vs ACT drain
                    acc_a = pspool.tile([128, XW * C // 2], f32, tag="acca")
                    acc_b = pspool.tile([128, XW * C // 2], f32, tag="accb")
                    acch = [acc_a, acc_b]
                    for dy in range(K):
                        xs = xpool.tile([128, XW + 2 * PAD, C], f16, tag="xs")
                        nc.sync.dma_start(
                            out=xs[:],
                            in_=xp[
                                yt * 128 + dy : yt * 128 + dy + 128,
                                x0 : x0 + XW + 2 * PAD,
                                :,
                            ],
                        )
                        for dx in range(K):
                            t = dy * K + dx
                            w_ap = (
                                kv2[:, t, :]
                                .rearrange("p (x two) -> p x two", two=2)
                                .unsqueeze(2)
                                .broadcast_to([128, XW, C // 2, 2])
                            )
                            prod = ppool.tile([128, XW * C], f16, tag="prod")
                            if t in T_ACT:
                                ai = T_ACT.index(t)
                                pv = prod.rearrange("p (x c) -> p x c", c=C)
                                for x in range(XW):
                                    nc.scalar.mul(
                                        pv[:, x, :],
                                        xs[:, dx + x, :],
                                        wma_sb[:, ai, x : x + 1],
                                    )
                            else:
                                eng = nc.gpsimd if dx == 2 else nc.vector
                                eng.tensor_tensor(
                                    out=prod.rearrange("p (x c) -> p x c", c=C),
                                    in0=xs[:, dx : dx + XW, :],
                                    in1=w_ap,
                                    op=Alu.mult,
                                )
                            for j in range(NSL):
                                nc.tensor.matmul(
                                    acch[j // (NSL // 2)][
                                        :,
                                        (j % (NSL // 2)) * 512 : (j % (NSL // 2) + 1)
                                        * 512,
                                    ],
                                    id_sb[:],
                                    prod[:, j * 512 : (j + 1) * 512],
                                    start=(t == 0),
                                    stop=(t == NT - 1),
                                )
                    accs = apool.tile([128, XW * C], f16, tag="accs")
                    for h in range(2):
                        nc.scalar.copy(
                            out=accs[:, h * (XW * C // 2) : (h + 1) * (XW * C // 2)],
                            in_=acch[h][:],
                        )
                    nc.sync.dma_start(
                        out=y_out[yt * 128 : (yt + 1) * 128, x0 : x0 + XW, :],
                        in_=accs.rearrange("p (x c) -> p x c", c=C),
                    )

    nc.finalize()
    return nc


def _get_nc():
    if "nc" not in _CACHE:
        _CACHE["nc"] = _build_nc()
    return _CACHE["nc"]


def _prep_core_inputs(input, kernel_bank, buckets):
    """Host-side prep: fp16 channel-minor padded image + duplicated weight map."""
    # x: [B, C, H, W] f32 -> per-core [HP, WP, C] f16
    x16 = input.astype(np.float16)
    xt = np.ascontiguousarray(x16.transpose(0, 2, 3, 1))  # [B, H, W, C]
    xpad = np.pad(xt, ((0, 0), (PAD, PAD), (PAD, PAD), (0, 0)))

    # weight map: [B, H, W] buckets -> [B, H, NT, 2W] f16 (x-pair duplicated)
    bank2 = kernel_bank.reshape(NB, NT).astype(np.float16)
    kv = bank2[buckets]  # [B, H, W, NT] f16
    kv = kv.transpose(0, 1, 3, 2)  # [B, H, NT, W]
    wm2 = np.repeat(kv, 2, axis=3)  # [B, H, NT, 2W]
    wm2 = np.ascontiguousarray(wm2)
    # fp32 planes of the ACT-engine taps' weights: [B, H, n_act, W]
    bank_act = kernel_bank.reshape(NB, NT)[:, list(T_ACT)].astype(np.float32)
    wma = bank_act[buckets]  # [B, H, W, n_act]
    wma = np.ascontiguousarray(wma.transpose(0, 1, 3, 2))  # [B, H, n_act, W]
    return xpad, wm2, wma


def kernel(input, kernel_bank, buckets):
    from concourse.bass_utils import run_bass_kernel_spmd

    nc = _get_nc()

    xpad, wm2, wma = _prep_core_inputs(
        np.asarray(input), np.asarray(kernel_bank), np.asarray(buckets)
    )
    ident = np.eye(128, dtype=np.float16)

    in_maps = [
        {"xp": xpad[i], "wm2": wm2[i], "wma": wma[i], "ident": ident}
        for i in range(N_CORES)
    ]
    res = run_bass_kernel_spmd(nc, in_maps, list(range(N_CORES)))
    # device output is [H, W, C] f16; back to [C, H, W] f32
    out = np.stack(
        [
            res.results[i]["y"].astype(np.float32).transpose(2, 0, 1)
            for i in range(N_CORES)
        ],
        axis=0,
    )
    return np.ascontiguousarray(out, dtype=np.float32)
